# revision 47
# baseline (speedup 1.0000x reference)
"""Trainium2 Bass kernel for nn_AttentionModel (patch-transformer + MSE loss).

Math (per batch element b of B=32), via the baseline's algebraic fold:
    Xa       = [normalized patches^T ; ones]      [33, T=1024]
    scores^T = Xa^T (M_qk^T Xa)  in [s, t] layout; exp/16, causal
    pred_u   = VW_aug^T exp(...); row 32 = softmax denominator (css)
    loss    += sum((pred_u/css - next patches)^2)

Sharding: data-parallel, 4 batch elements per core x 8 cores; host sums
the per-core partials.

Performance structure (v15, ~84us vs 96us baseline):
  * batch-PAIR packing: batches (A, B) of a pair live at partitions
    0-32 / 64-96; all K=33 / M=33 matmuls (score, PV, Y, VW, broadcasts)
    issue as two instructions on disjoint PE quadrants (tile_position
    auto-derived from base partitions) and run CONCURRENTLY.  Concurrent
    full-partition MMs must target different PSUM banks (write-port
    conflict wedges the device) -- see the VW bank split.
  * pu checkerboard (A,h0)->bank0, (B,h0)->bank1, (A,h1)->bank1,
    (B,h1)->bank0: no PSUM bank ever hosts two interleaved accumulation
    groups, B's t-columns are rotated by 512 (un-rotated in the tail).
  * exp covers both batches per instruction via 2-bank rect APs; the
    diagonal-block causal mask is a DVE multiply with a doubled triu.
  * transposes: two whole-tile [128,128] PE transposes per batch
    (transpose outputs must start at PSUM partition 0), one fused
    normalize, then 4 regroup DMAs per batch scatter the (kc,ps)
    interleaved rows into token order (engines cannot cross partitions;
    DMA dispatch costs ~0.6-1.2us of HWDGE sequencer time each, so few
    big DMAs beat many small ones).
  * 1/css runs on DVE in a DMA-gathered [128, 8] staging layout
    (engine cost is free-size-bound, so [1, 512] row ops are poison);
    the final half uses ScalarE Ln/Exp directly since the ~4us DMA
    round-trip latency cannot be hidden there.
  * software-pipelined emission: engine FIFOs execute in program order,
    so the pair-1 prologue is emitted inside pair-0's exp stream, and
    epilogue PE work (bcast MMs) is emitted only after its recip chain
    is guaranteed complete (a waiting MM head-of-line blocks the PE).
  * stats are vectorized [1, 2]-per-step chains per pair, fed by
    per-batch sums that overlap the x DMA loads.
ScalarE exp (~18.4k causal columns -> ~15.4us minimum) and the cold
(1.2 GHz) PE stream pace the steady state; the HAM clock gate never
re-engages inside the dependency-broken stream.
"""

import math

import numpy as np

import concourse.bass as bass
import concourse.mybir as mybir
import concourse.tile as tile
from concourse.bass_utils import run_bass_kernel_spmd
from concourse.masks import make_identity, make_upper_triangular
from concourse.vector_clock import ScopedClock

F32 = mybir.dt.float32
BF16 = mybir.dt.bfloat16
AX = mybir.AxisListType
ALU = mybir.AluOpType
AF = mybir.ActivationFunctionType

N_CORES = 8
B = 32
L = 32768
PS = 32
D = 256
T = L // PS  # 1024
BPC = B // N_CORES  # batch elements per core = 4
NT = T // 128  # 8 s-tiles
KA = PS + 1  # augmented contraction dim (extra ones row)
SCALE = 1.0 / math.sqrt(D)  # 1/16
PB = 64  # partition base of batch B within a pair


class SplitDrainTileContext(tile.TileContext):
    """TileContext whose final drain splits sem waits across multiple drain
    instructions -- this walrus rejects >1 sync wait per instruction."""

    def _drain_and_barrier(self, tick_clock, wait_clock):
        probe = mybir.InstDrain(name=f"I-{self.nc.next_id()}", ins=[], outs=[])
        probe.engine = mybir.EngineType.SP
        wait_clock.add_sem_waits(probe, ScopedClock({None: tick_clock.global_clock}))
        waits = list(probe.sync_info.on_wait) if probe.sync_info else []
        assert self.sems is not None
        handles = {h.num: h for h in self.sems.allocated().values()}
        if not waits:
            self.nc.sync.drain()
        for w in waits:
            d = self.nc.sync.drain()
            d.wait_op(handles[w.id], w.wait_value, "sem-ge", check=False)
        self.nc.all_engine_barrier()
        popped = self.nc._tile_sem_poison_stack.pop()
        assert popped is self._sem_poison
        self.nc.clear_and_free_semaphores(list(self.sems.allocated().values()))
        self.nc.all_engine_barrier()


def split_excess_waits(nc, max_waits=1):
    """This walrus rejects instructions carrying more than one sync wait.
    Hoist extra waits onto the immediately preceding same-engine
    instruction when that instruction signals nothing, else insert a
    wait-only drain."""
    for f in nc.m.functions:
        for blk in f.blocks:
            insts = list(blk.instructions)
            out = []
            prev_by_engine = {}
            changed = False
            for inst in insts:
                si = inst.sync_info
                waits = list(si.on_wait) if si else []
                if len(waits) > max_waits:
                    changed = True
                    extra, keep = waits[:-max_waits], waits[-max_waits:]
                    remaining = []
                    prev = prev_by_engine.get(str(inst.engine))
                    for w in extra:
                        psi = prev.sync_info if prev is not None else None
                        if prev is not None and (
                            psi is None
                            or (len(psi.on_wait) == 0 and len(psi.on_update) == 0)
                        ):
                            prev.sync_info = mybir.SyncInfo(on_wait=[w], on_update=[])
                            prev = None  # one hoist per predecessor
                        else:
                            remaining.append(w)
                    for w in remaining:
                        dr = mybir.InstDrain(name=f"I-{nc.next_id()}", ins=[], outs=[])
                        dr.engine = inst.engine
                        dr.sync_info = mybir.SyncInfo(on_wait=[w], on_update=[])
                        out.append(dr)
                    inst.sync_info = mybir.SyncInfo(
                        on_wait=keep, on_update=list(si.on_update)
                    )
                out.append(inst)
                prev_by_engine[str(inst.engine)] = inst
            if changed:
                blk.instructions = out


def dedupe_ldweights(nc):
    """Drop an InstLdweights whose operand AP is byte-identical to the
    immediately preceding PE instruction's InstLdweights (no other PE
    instruction between them) -- the stationary operand is still loaded.
    Only legal when the elided load carries no sync actions."""
    for f in nc.m.functions:
        for blk in f.blocks:
            insts = list(blk.instructions)
            out = []
            last_pe_ldw_key = None
            changed = False
            for inst in insts:
                if str(inst.engine) != "EngineType.PE":
                    out.append(inst)
                    continue
                tname = type(inst).__name__
                if tname == "InstLdweights":
                    si = inst.sync_info
                    has_sync = si and (len(si.on_wait) or len(si.on_update))
                    try:
                        key = str(inst.ins[0])
                    except Exception:
                        key = None
                    if key is not None and key == last_pe_ldw_key and not has_sync:
                        changed = True
                        continue  # elide duplicate load
                    last_pe_ldw_key = key
                    out.append(inst)
                else:
                    if tname == "InstMatmult":
                        if getattr(inst, "is_transpose", None):
                            last_pe_ldw_key = None
                    else:
                        last_pe_ldw_key = None
                    out.append(inst)
            if changed:
                blk.instructions = out


def build_program():
    nc = bass.Bass("TRN2", target_bir_lowering=False, debug=False, num_devices=N_CORES)

    x_d = nc.dram_tensor("x", [BPC, L], F32, kind="ExternalInput")
    mqk_d = nc.dram_tensor("m_qk", [KA, KA], BF16, kind="ExternalInput")
    mvo_d = nc.dram_tensor("m_vo", [KA, KA], BF16, kind="ExternalInput")
    out_d = nc.dram_tensor("loss_partial", [1, 1], F32, kind="ExternalOutput")

    from contextlib import ExitStack

    with SplitDrainTileContext(nc) as tc, ExitStack() as ctx:
        cpool = ctx.enter_context(tc.tile_pool(name="consts", bufs=1))
        # PSUM: rotating pool (2x [128,1024] = 4 banks) for transient tiles;
        # persistent pool for pred_u + the small stats tiles (4 banks).
        prot = ctx.enter_context(tc.tile_pool(name="prot", bufs=2, space="PSUM"))
        ppu = ctx.enter_context(tc.tile_pool(name="ppu", bufs=2, space="PSUM"))
        xpool = ctx.enter_context(tc.tile_pool(name="xc", bufs=4))
        spool = ctx.enter_context(tc.tile_pool(name="small", bufs=8))
        bigpool = ctx.enter_context(tc.tile_pool(name="big", bufs=2))
        epool = ctx.enter_context(tc.tile_pool(name="et", bufs=3))
        scratch = ctx.enter_context(tc.tile_pool(name="scratch", bufs=2))

        # ---- constants ----
        ident_f = cpool.tile([128, 128], F32)
        make_identity(nc, ident_f[:])
        ident_b = cpool.tile([128, 128], BF16)
        make_identity(nc, ident_b[:])
        # doubled keep-mask (upper incl diag) for the DVE diagonal-block
        # mask of both batches at once
        triu2 = cpool.tile([128, 256], BF16)
        make_upper_triangular(nc, triu2[:, 0:128], val=1.0, diag=True)
        make_upper_triangular(nc, triu2[:, 128:256], val=1.0, diag=True)
        ones_col = cpool.tile([128, 1], F32)
        nc.vector.memset(ones_col[:], 1.0)
        ones_row = cpool.tile([1, PS], F32)
        nc.vector.memset(ones_row[:], 1.0)
        ones_t = cpool.tile([128, PS], BF16)
        nc.vector.memset(ones_t[:], 1.0)

        mqk2 = cpool.tile([128, KA], BF16)
        nc.gpsimd.dma_start(mqk2[0:KA, :], mqk_d.ap()[:])
        nc.gpsimd.dma_start(mqk2[PB : PB + KA, :], mqk_d.ap()[:])
        mvo2 = cpool.tile([128, KA], BF16)
        nc.gpsimd.dma_start(mvo2[0:KA, :], mvo_d.ap()[:])
        nc.gpsimd.dma_start(mvo2[PB : PB + KA, :], mvo_d.ap()[:])

        lp_all = cpool.tile([128, 4], F32)  # per-(pair, half) loss partials
        nc.vector.memset(lp_all[:], 0.0)


        # ---- x loads + per-batch sums + per-pair stats chains ----
        xcall = xpool.tile([128, BPC * (L // 128)], F32, name="xcall")
        CB = L // 128  # 256 cols per batch
        xcs = [xcall[:, b * CB : (b + 1) * CB] for b in range(BPC)]
        sums = spool.tile([128, 2 * BPC], F32, tag="sums")
        for b in range(BPC):
            # partition u, free (k, ps) <- x[b, (128k + u)*32 + ps]
            qeng = nc.sync if b % 2 == 0 else nc.scalar
            qeng.dma_start(
                xcall[:, b * CB : (b + 1) * CB].rearrange(
                    "u (k ps) -> u k ps", ps=PS
                ),
                x_d.ap()[b].rearrange("(k u ps) -> u k ps", u=128, ps=PS),
            )
            sc_col = 4 * (b // 2) + (b % 2)  # pair-contiguous: sA sB qA qB
            nc.vector.reduce_sum(sums[:, sc_col : sc_col + 1], xcs[b], axis=AX.X)
            sq_scr = scratch.tile([128, CB], F32, tag="sq", name=f"sq_{b}")
            nc.vector.tensor_tensor(out=sq_scr[:], in0=xcs[b], in1=xcs[b], op=ALU.mult)
            nc.vector.reduce_sum(
                sums[:, sc_col + 2 : sc_col + 3], sq_scr[:], axis=AX.X
            )

        # force the ACT table load (Ln/Exp set) right after the scalar
        # queue's x-load dispatches -- self-read, so it neither waits on the
        # lp memset nor delays the HWDGE dispatches ahead of it
        actw = cpool.tile([1, 2], F32)
        nc.scalar.activation(actw[:], actw[:], AF.Exp)

        scqs = []
        for p in range(2):
            # tot = (sumA, sumB, sqA, sqB) for this pair
            tot_ps = ppu.tile([1, 4], F32, tag="pu", name=f"totps_{p}")
            nc.tensor.matmul(
                tot_ps[:], ones_col[:], sums[:, 4 * p : 4 * p + 4],
                start=True, stop=True, skip_group_check=True,
            )
            tot = spool.tile([1, 4], F32, tag="tot", name=f"tot_{p}")
            nc.vector.tensor_copy(tot[:], tot_ps[:])
            w = spool.tile([1, 12], F32, tag="w", name=f"w_{p}")
            scq = spool.tile([1, 4], F32, tag="scq", name=f"scq_{p}")
            q_r = scq[:].rearrange("p (b q) -> p b q", q=2)[:, :, 0:1]
            q_s = scq[:].rearrange("p (b q) -> p b q", q=2)[:, :, 1:2]
            nc.scalar.mul(w[:, 0:2], tot[:, 0:2], 1.0 / L)  # mean
            nc.vector.tensor_tensor(
                out=w[:, 2:4], in0=tot[:, 0:2], in1=w[:, 0:2], op=ALU.mult
            )
            nc.vector.tensor_tensor(
                out=w[:, 4:6], in0=tot[:, 2:4], in1=w[:, 2:4], op=ALU.subtract
            )
            nc.scalar.activation(w[:, 6:8], w[:, 4:6], AF.Ln, scale=1.0 / (L - 1))
            # rstd = exp(-ln(var)/2); dropping the reference's +1e-5 on std
            # shifts rstd by 1e-5 relative -- far below the bf16 noise
            nc.scalar.activation(q_r, w[:, 6:8], AF.Exp, scale=-0.5)
            nc.scalar.mul(w[:, 0:2], w[:, 0:2], -1.0)  # -mean
            nc.vector.tensor_tensor(out=q_s, in0=w[:, 0:2], in1=q_r, op=ALU.mult)
            scqs.append(scq)

        ones_r128 = cpool.tile([1, 128], F32)
        nc.vector.memset(ones_r128[:], 1.0)
        bcp = []
        for p in range(2):
            # (rstd, shift) broadcast to all 128 partitions; A cols 0:2,
            # B cols 2:4 (the transposed layout mixes batches by column,
            # so each batch needs its scalars on every partition)
            bc_ps = ppu.tile([128, 4], F32, tag="pu", name=f"bcps_{p}")
            nc.tensor.matmul(
                bc_ps[:, 0:2], ones_r128[:], scqs[p][:, 0:2],
                start=True, stop=True, skip_group_check=True,
            )
            nc.tensor.matmul(
                bc_ps[:, 2:4], ones_r128[:], scqs[p][:, 2:4],
                start=True, stop=True, skip_group_check=True,
            )
            bc = spool.tile([128, 4], F32, tag="bc", name=f"bc_{p}")
            nc.vector.tensor_copy(bc[:], bc_ps[:])
            bcp.append(bc)

        # ---- per-pair state ----
        xnt = [None, None]
        y = [None, None]
        vw = [None, None]
        pu = [None, None]
        rr = [None, None]
        bcrs = [None, None]
        predt = [None, None]
        dds = [None, None]

        def prologue_tp(p):
            """Two whole-tile [128,128] PE transposes per batch (legal:
            full-partition output at base 0), one normalize per batch, then
            4 regroup DMAs per batch scatter the (kc,ps)-interleaved rows
            into xnt token order."""
            bc = bcp[p]
            xnt[p] = bigpool.tile([128, T], BF16, tag="xnt", name=f"xnt_{p}")
            xt = xnt[p]
            nc.vector.memset(xt[PS : PS + 1, :], 1.0)
            nc.vector.memset(xt[PB + PS : PB + PS + 1, :], 1.0)
            tp_ps = prot.tile([128, 512], F32, tag="rot", name=f"tp_{p}")
            for bi in range(2):
                for h in range(2):
                    nc.tensor.transpose(
                        tp_ps[:, 256 * bi + 128 * h : 256 * bi + 128 * h + 128],
                        xcs[2 * p + bi][:, 128 * h : 128 * h + 128],
                        ident_f[:],
                    )
            stag = scratch.tile([128, 512], BF16, tag="stag", name=f"stag_{p}")
            for bi in range(2):
                nc.vector.tensor_scalar(
                    out=stag[:, 256 * bi : 256 * bi + 256],
                    in0=tp_ps[:, 256 * bi : 256 * bi + 256],
                    scalar1=bc[:, 2 * bi : 2 * bi + 1],
                    scalar2=bc[:, 2 * bi + 1 : 2 * bi + 2],
                    op0=ALU.mult,
                    op1=ALU.add,
                )
                # row 32*kc+ps, col (h,u) of this batch block holds token
                # 128*(4h+kc)+u elem ps -> xnt[row, 128*(4h+kc)+u]
                row0 = 0 if bi == 0 else PB
                for kc in range(4):
                    qeng = (nc.sync, nc.scalar, nc.gpsimd, nc.sync)[kc]
                    qeng.dma_start(
                        xt[row0 : row0 + PS, :]
                        .rearrange("p (h k u) -> p h k u", k=4, u=128)[:, :, kc, :],
                        stag[
                            32 * kc : 32 * kc + PS, 256 * bi : 256 * bi + 256
                        ].rearrange("p (h u) -> p h u", u=128),
                    )

        def prologue_yvw(p):
            xt = xnt[p]
            # Y = M_qk^T Xa, pair-concurrent
            y[p] = bigpool.tile([128, T], BF16, tag="y", name=f"y_{p}")
            for n in range(2):
                y_ps = prot.tile([128, 512], F32, tag="rot", name=f"yps_{p}_{n}")
                nc.tensor.matmul(
                    y_ps[0:KA, :], mqk2[0:KA, :],
                    xt[0:KA, n * 512 : (n + 1) * 512],
                    start=True, stop=True, skip_group_check=True,
                )
                nc.tensor.matmul(
                    y_ps[PB : PB + KA, :], mqk2[PB : PB + KA, :],
                    xt[PB : PB + KA, n * 512 : (n + 1) * 512],
                    start=True, stop=True, skip_group_check=True,
                )
                nc.vector.tensor_copy(
                    y[p][0 : PB + KA, n * 512 : (n + 1) * 512], y_ps[0 : PB + KA, :]
                )

            # VW = Xa^T M_vo_aug: A_j in bank0 at 64j, B_j in bank1 (two
            # concurrent full-partition MMs must not share a PSUM bank)
            vw_ps = prot.tile([128, 1024], F32, tag="rot", name=f"vwps_{p}")
            for j in range(NT):
                nc.tensor.matmul(
                    vw_ps[:, 64 * j : 64 * j + KA],
                    xt[0:KA, j * 128 : (j + 1) * 128],
                    mvo2[0:KA, :],
                    start=True, stop=True, skip_group_check=True,
                )
                nc.tensor.matmul(
                    vw_ps[:, 512 + 64 * j : 512 + 64 * j + KA],
                    xt[PB : PB + KA, j * 128 : (j + 1) * 128],
                    mvo2[PB : PB + KA, :],
                    start=True, stop=True, skip_group_check=True,
                )
            # vw cols: A_j at 33j, B_j at 264+33j
            vw[p] = bigpool.tile([128, NT * 2 * KA], BF16, tag="vw", name=f"vw_{p}")
            nc.vector.tensor_copy(
                vw[p][:].rearrange("u (s e) -> u s e", e=KA),
                vw_ps[:].rearrange("u (s e) -> u s e", e=64)[:, :, 0:KA],
            )
            pu[p] = ppu.tile([128, 1024], F32, tag="pu", name=f"pu_{p}")
            rr[p] = scratch.tile([128, 1024], BF16, tag="rr", name=f"rr_{p}")
            bcrs[p] = scratch.tile([128, 1024], F32, tag="bcr", name=f"bcr_{p}")
            predt[p] = scratch.tile([128, 1024], BF16, tag="predt", name=f"predt_{p}")
            dds[p] = scratch.tile([128, 1024], BF16, tag="dd", name=f"dd_{p}")
            nc.gpsimd.memset(dds[p][PS:PB, 0 : T - 1], 0.0)

        def main_half(p, n):
            """scores -> exp -> PV for one t-half, pair-concurrent.
            pu checkerboard: (A,h) -> bank h, (B,h) -> bank 1-h."""
            xt, yp, vwp, pup = xnt[p], y[p], vw[p], pu[p]
            nj = 4 * n + 4
            bcol = (1 - n) * 512
            for j in range(nj):
                off = max(0, j * 128 - n * 512)
                diag = j * 128 >= n * 512
                sc_ps = prot.tile(
                    [128, 1024], F32, tag="rot", name=f"scps_{p}_{n}_{j}"
                )
                nc.tensor.matmul(
                    sc_ps[:, off:512],
                    xt[0:KA, j * 128 : (j + 1) * 128],
                    yp[0:KA, n * 512 + off : (n + 1) * 512],
                    start=True, stop=True, skip_group_check=True,
                )
                nc.tensor.matmul(
                    sc_ps[:, 512 + off : 1024],
                    xt[PB : PB + KA, j * 128 : (j + 1) * 128],
                    yp[PB : PB + KA, n * 512 + off : (n + 1) * 512],
                    start=True, stop=True, skip_group_check=True,
                )
                et = epool.tile([128, 1024], BF16, tag="et", name=f"et_{p}_{n}_{j}")
                nc.scalar.activation(
                    et[:].rearrange("u (b c) -> u b c", b=2)[:, :, off:512],
                    sc_ps[:].rearrange("u (b c) -> u b c", b=2)[:, :, off:512],
                    AF.Exp,
                    scale=SCALE,
                )
                if diag:
                    # zero the s > t half of the diagonal block (DVE has
                    # slack in the stream; keeps the PE free)
                    db = et[:].rearrange("u (b c) -> u b c", b=2)[
                        :, :, off : off + 128
                    ]
                    nc.vector.tensor_tensor(
                        out=db, in0=db,
                        in1=triu2[:].rearrange("u (b c) -> u b c", b=2),
                        op=ALU.mult,
                    )
                nc.tensor.matmul(
                    pup[0:KA, n * 512 + off : (n + 1) * 512],
                    vwp[:, j * KA : (j + 1) * KA],
                    et[:, off:512],
                    start=(j == 0), stop=(j == nj - 1), skip_group_check=True,
                )
                nc.tensor.matmul(
                    pup[PB : PB + KA, bcol + off : bcol + 512],
                    vwp[:, NT * KA + j * KA : NT * KA + (j + 1) * KA],
                    et[:, 512 + off : 1024],
                    start=(j == 0), stop=(j == nj - 1), skip_group_check=True,
                )

        def epiA(p, n):
            """1/colsum for one half: css rows (32, A-cols) and (96, B-cols)
            DMA-gathered to a [128, 8] layout, DVE reciprocal, DMA back."""
            pup = pu[p]
            bcol = (1 - n) * 512
            csb = scratch.tile([128, 1024], F32, tag="lnr", name=f"csb_{p}_{n}")
            nc.vector.tensor_copy(csb[0 : PB + PS + 1, :], pup[0 : PB + PS + 1, :])
            stg = spool.tile([128, 8], F32, tag="stg", name=f"stg_{p}_{n}")
            nc.sync.dma_start(
                stg[:, 0:4].rearrange("p q -> p () q"),
                csb[PS : PS + 1, n * 512 : (n + 1) * 512].rearrange(
                    "p (a q) -> p a q", q=4
                ),
            )
            nc.scalar.dma_start(
                stg[:, 4:8].rearrange("p q -> p () q"),
                csb[PB + PS : PB + PS + 1, bcol : bcol + 512].rearrange(
                    "p (a q) -> p a q", q=4
                ),
            )
            rstg = spool.tile([128, 8], F32, tag="rstg", name=f"rstg_{p}_{n}")
            nc.vector.reciprocal(rstg[:], stg[:])
            rb16 = spool.tile([128, 8], BF16, tag="rb16", name=f"rb16_{p}_{n}")
            nc.vector.tensor_copy(rb16[:], rstg[:])
            rrp = rr[p]
            nc.sync.dma_start(
                rrp[PS : PS + 1, n * 512 : (n + 1) * 512].rearrange(
                    "p (a q) -> p a q", q=4
                ),
                rb16[:, 0:4].rearrange("p q -> p () q"),
            )
            nc.scalar.dma_start(
                rrp[PB + PS : PB + PS + 1, bcol : bcol + 512].rearrange(
                    "p (a q) -> p a q", q=4
                ),
                rb16[:, 4:8].rearrange("p q -> p () q"),
            )
        def epiA_scalar(p, n):
            """Same as epiA but via ScalarE Ln/Exp straight from PSUM --
            no DMA round-trip latency; used for the final half where the
            DMA latency cannot be hidden."""
            pup = pu[p]
            bcol = (1 - n) * 512
            rrp = rr[p]
            lnr = scratch.tile([128, 1024], F32, tag="lnr", name=f"lnr_{p}_{n}")
            # single-row ops: ACT cost is free-size-bound, and touching only
            # the css rows avoids a false WAW against the other half's rr
            # rows (which used to stall its broadcast MMs ~3.5us)
            nc.scalar.activation(
                lnr[PS : PS + 1, n * 512 : (n + 1) * 512],
                pup[PS : PS + 1, n * 512 : (n + 1) * 512],
                AF.Ln,
            )
            nc.scalar.activation(
                rrp[PS : PS + 1, n * 512 : (n + 1) * 512],
                lnr[PS : PS + 1, n * 512 : (n + 1) * 512],
                AF.Exp, scale=-1.0,
            )
            nc.scalar.activation(
                lnr[PB + PS : PB + PS + 1, bcol : bcol + 512],
                pup[PB + PS : PB + PS + 1, bcol : bcol + 512],
                AF.Ln,
            )
            nc.scalar.activation(
                rrp[PB + PS : PB + PS + 1, bcol : bcol + 512],
                lnr[PB + PS : PB + PS + 1, bcol : bcol + 512],
                AF.Exp, scale=-1.0,
            )

        def epiB(p, n):
            """PE broadcast of 1/css + evacuation to sbuf (emitted late so
            the PE FIFO never blocks on the recip DMA chain)."""
            bcol = (1 - n) * 512
            rrp = rr[p]
            bcr_ps = prot.tile([128, 1024], F32, tag="rot", name=f"bcrps_{p}_{n}")
            nc.tensor.matmul(
                bcr_ps[0:PS, n * 512 : (n + 1) * 512],
                ones_t[PS : PS + 1, :],
                rrp[PS : PS + 1, n * 512 : (n + 1) * 512],
                start=True, stop=True, skip_group_check=True,
            )
            nc.tensor.matmul(
                bcr_ps[PB : PB + PS, bcol : bcol + 512],
                ones_t[PB + PS : PB + PS + 1, :],
                rrp[PB + PS : PB + PS + 1, bcol : bcol + 512],
                start=True, stop=True, skip_group_check=True,
                tile_position=(PB + PS, PB),
            )
            ceng = nc.scalar if (p, n) in ((0, 1), (1, 1)) else None
            if ceng is not None:
                ceng.copy(
                    bcrs[p][0:PS, n * 512 : (n + 1) * 512],
                    bcr_ps[0:PS, n * 512 : (n + 1) * 512],
                )
                ceng.copy(
                    bcrs[p][PB : PB + PS, bcol : bcol + 512],
                    bcr_ps[PB : PB + PS, bcol : bcol + 512],
                )
            else:
                nc.vector.tensor_copy(
                    bcrs[p][0:PS, n * 512 : (n + 1) * 512],
                    bcr_ps[0:PS, n * 512 : (n + 1) * 512],
                )
                nc.vector.tensor_copy(
                    bcrs[p][PB : PB + PS, bcol : bcol + 512],
                    bcr_ps[PB : PB + PS, bcol : bcol + 512],
                )

        def tail_half(p, n):
            """pred = pu/css and squared-error partial for one t-half.
            dd col c holds t=c; the B rows read checkerboarded pred cols."""
            pup, xt, pt, dd = pu[p], xnt[p], predt[p], dds[p]
            bcol = (1 - n) * 512
            nc.vector.tensor_tensor(
                out=pt[0:PS, n * 512 : (n + 1) * 512],
                in0=pup[0:PS, n * 512 : (n + 1) * 512],
                in1=bcrs[p][0:PS, n * 512 : (n + 1) * 512],
                op=ALU.mult,
            )
            nc.vector.tensor_tensor(
                out=pt[PB : PB + PS, bcol : bcol + 512],
                in0=pup[PB : PB + PS, bcol : bcol + 512],
                in1=bcrs[p][PB : PB + PS, bcol : bcol + 512],
                op=ALU.mult,
            )
            # dd cols for this half: t in [512n, 512n+512) (clip t=1023)
            c0 = n * 512
            c1 = min((n + 1) * 512, T - 1)
            deng = nc.vector if (p, n) == (1, 1) else nc.gpsimd
            deng.tensor_tensor(
                out=dd[0:PS, c0:c1],
                in0=pt[0:PS, c0:c1],
                in1=xt[0:PS, c0 + 1 : c1 + 1],
                op=ALU.subtract,
            )
            deng.tensor_tensor(
                out=dd[PB : PB + PS, c0:c1],
                in0=pt[PB : PB + PS, bcol : bcol + (c1 - c0)],
                in1=xt[PB : PB + PS, c0 + 1 : c1 + 1],
                op=ALU.subtract,
            )
            nc.scalar.activation(
                dd[0 : PB + PS, c0:c1],
                dd[0 : PB + PS, c0:c1],
                AF.Square,
                accum_out=lp_all[0 : PB + PS, 2 * p + n : 2 * p + n + 1],
            )

        # ---- software-pipelined emission ----
        prologue_tp(0)
        prologue_tp(1)
        prologue_yvw(0)
        prologue_yvw(1)
        main_half(0, 0)
        epiA(0, 0)
        main_half(0, 1)
        epiA(0, 1)
        main_half(1, 0)
        epiA(1, 0)
        epiB(0, 0)
        tail_half(0, 0)
        main_half(1, 1)
        epiB(0, 1)
        tail_half(0, 1)
        epiA_scalar(1, 1)
        epiB(1, 0)
        tail_half(1, 0)
        epiB(1, 1)
        tail_half(1, 1)

        # ---- final: total partial over pairs & partitions ----
        lsum = spool.tile([128, 1], F32)
        nc.vector.reduce_sum(lsum[:], lp_all[:], axis=AX.X)
        tot_ps2 = prot.tile([1, 1], F32, tag="rot")
        nc.tensor.matmul(tot_ps2[:], ones_col[:], lsum[:], start=True, stop=True)
        out_sb = spool.tile([1, 1], F32)
        nc.vector.tensor_copy(out_sb[:], tot_ps2[:])
        nc.sync.dma_start(out_d.ap()[:], out_sb[:])

    split_excess_waits(nc)
    dedupe_ldweights(nc)
    return nc


_program_cache = {}


def _get_program():
    if "nc" not in _program_cache:
        _program_cache["nc"] = build_program()
    return _program_cache["nc"]


def make_in_maps(x, W_proj, b_proj, W_qkv, b_qkv, W_out, b_out, W_head, b_head):
    import ml_dtypes

    f8 = np.float64
    w_eff = W_proj.astype(f8) @ W_qkv.astype(f8)  # [32, 768]
    b_eff = b_proj.astype(f8) @ W_qkv.astype(f8) + b_qkv.astype(f8)  # [768]
    w_aug = np.concatenate([w_eff, b_eff[None, :]], axis=0)  # [33, 768]
    wq, wk, wv = w_aug[:, 0:D], w_aug[:, D : 2 * D], w_aug[:, 2 * D : 3 * D]
    m_qk = wq @ wk.T  # [33, 33]
    w_oh = W_out.astype(f8) @ W_head.astype(f8)  # [256, 32]
    b_oh = b_out.astype(f8) @ W_head.astype(f8) + b_head.astype(f8)  # [32]
    m_vo = wv @ w_oh  # [33, 32]
    m_vo[PS, :] += b_oh
    e_ones = np.zeros((KA, 1), f8)
    e_ones[PS, 0] = 1.0  # selects Xa's ones row -> colsum output column
    m_vo_aug = np.concatenate([m_vo, e_ones], axis=1)  # [33, 33]

    mqk_b = np.ascontiguousarray(m_qk.astype(ml_dtypes.bfloat16))
    mvo_b = np.ascontiguousarray(m_vo_aug.astype(ml_dtypes.bfloat16))

    in_maps = []
    for core in range(N_CORES):
        xs = np.ascontiguousarray(x[core * BPC : (core + 1) * BPC])
        in_maps.append({"x": xs, "m_qk": mqk_b, "m_vo": mvo_b})
    return in_maps


def kernel(**inputs) -> np.ndarray:
    inputs = {k: np.asarray(v) for k, v in inputs.items()}
    nc = _get_program()
    in_maps = make_in_maps(**inputs)
    res = run_bass_kernel_spmd(nc, in_maps, core_ids=list(range(N_CORES)))
    total = sum(float(res.results[i]["loss_partial"][0, 0]) for i in range(N_CORES))
    loss = total / (B * (T - 1) * PS)
    return np.float32(loss)


if __name__ == "__main__":
    rng = np.random.default_rng(0)
    ins = {
        "x": rng.standard_normal((B, L)).astype(np.float32),
        "W_proj": (rng.standard_normal((PS, D)) / math.sqrt(PS)).astype(np.float32),
        "b_proj": np.zeros(D, np.float32),
        "W_qkv": (rng.standard_normal((D, 3 * D)) / math.sqrt(D)).astype(np.float32),
        "b_qkv": np.zeros(3 * D, np.float32),
        "W_out": (rng.standard_normal((D, D)) / math.sqrt(D)).astype(np.float32),
        "b_out": np.zeros(D, np.float32),
        "W_head": (rng.standard_normal((D, PS)) / math.sqrt(D)).astype(np.float32),
        "b_head": np.zeros(PS, np.float32),
    }
    got = kernel(**ins)
    print("kernel loss:", got)



# revision 48
# speedup vs baseline: 1.0209x; 1.0209x over previous
"""Trainium2 Bass kernel for nn_AttentionModel (patch-transformer + MSE loss).

Math (per batch element b of B=32), via the baseline's algebraic fold:
    Xa       = [normalized patches^T ; ones]      [33, T=1024]
    scores^T = Xa^T (M_qk^T Xa)  in [s, t] layout; exp/16, causal
    pred_u   = VW_aug^T exp(...); row 32 = softmax denominator (css)
    loss    += sum((pred_u/css - next patches)^2)

Sharding: data-parallel, 4 batch elements per core x 8 cores; host sums
the per-core partials.

Performance structure (v15, ~84us vs 96us baseline):
  * batch-PAIR packing: batches (A, B) of a pair live at partitions
    0-32 / 64-96; all K=33 / M=33 matmuls (score, PV, Y, VW, broadcasts)
    issue as two instructions on disjoint PE quadrants (tile_position
    auto-derived from base partitions) and run CONCURRENTLY.  Concurrent
    full-partition MMs must target different PSUM banks (write-port
    conflict wedges the device) -- see the VW bank split.
  * pu checkerboard (A,h0)->bank0, (B,h0)->bank1, (A,h1)->bank1,
    (B,h1)->bank0: no PSUM bank ever hosts two interleaved accumulation
    groups, B's t-columns are rotated by 512 (un-rotated in the tail).
  * exp covers both batches per instruction via 2-bank rect APs; the
    diagonal-block causal mask is a DVE multiply with a doubled triu.
  * transposes: two whole-tile [128,128] PE transposes per batch
    (transpose outputs must start at PSUM partition 0), one fused
    normalize, then 4 regroup DMAs per batch scatter the (kc,ps)
    interleaved rows into token order (engines cannot cross partitions;
    DMA dispatch costs ~0.6-1.2us of HWDGE sequencer time each, so few
    big DMAs beat many small ones).
  * 1/css runs on DVE in a DMA-gathered [128, 8] staging layout
    (engine cost is free-size-bound, so [1, 512] row ops are poison);
    the final half uses ScalarE Ln/Exp directly since the ~4us DMA
    round-trip latency cannot be hidden there.
  * software-pipelined emission: engine FIFOs execute in program order,
    so the pair-1 prologue is emitted inside pair-0's exp stream, and
    epilogue PE work (bcast MMs) is emitted only after its recip chain
    is guaranteed complete (a waiting MM head-of-line blocks the PE).
  * stats are vectorized [1, 2]-per-step chains per pair, fed by
    per-batch sums that overlap the x DMA loads.
ScalarE exp (~18.4k causal columns -> ~15.4us minimum) and the cold
(1.2 GHz) PE stream pace the steady state; the HAM clock gate never
re-engages inside the dependency-broken stream.
"""

import math

import numpy as np

import concourse.bass as bass
import concourse.mybir as mybir
import concourse.tile as tile
from concourse.bass_utils import run_bass_kernel_spmd
from concourse.masks import make_identity, make_upper_triangular
from concourse.vector_clock import ScopedClock

F32 = mybir.dt.float32
BF16 = mybir.dt.bfloat16
AX = mybir.AxisListType
ALU = mybir.AluOpType
AF = mybir.ActivationFunctionType

N_CORES = 8
B = 32
L = 32768
PS = 32
D = 256
T = L // PS  # 1024
BPC = B // N_CORES  # batch elements per core = 4
NT = T // 128  # 8 s-tiles
KA = PS + 1  # augmented contraction dim (extra ones row)
SCALE = 1.0 / math.sqrt(D)  # 1/16
PB = 64  # partition base of batch B within a pair


class SplitDrainTileContext(tile.TileContext):
    """TileContext whose final drain splits sem waits across multiple drain
    instructions -- this walrus rejects >1 sync wait per instruction."""

    def _drain_and_barrier(self, tick_clock, wait_clock):
        probe = mybir.InstDrain(name=f"I-{self.nc.next_id()}", ins=[], outs=[])
        probe.engine = mybir.EngineType.SP
        wait_clock.add_sem_waits(probe, ScopedClock({None: tick_clock.global_clock}))
        waits = list(probe.sync_info.on_wait) if probe.sync_info else []
        assert self.sems is not None
        handles = {h.num: h for h in self.sems.allocated().values()}
        if not waits:
            self.nc.sync.drain()
        for w in waits:
            d = self.nc.sync.drain()
            d.wait_op(handles[w.id], w.wait_value, "sem-ge", check=False)
        self.nc.all_engine_barrier()
        popped = self.nc._tile_sem_poison_stack.pop()
        assert popped is self._sem_poison
        self.nc.clear_and_free_semaphores(list(self.sems.allocated().values()))
        self.nc.all_engine_barrier()


def split_excess_waits(nc, max_waits=1):
    """This walrus rejects instructions carrying more than one sync wait.
    Hoist extra waits onto the immediately preceding same-engine
    instruction when that instruction signals nothing, else insert a
    wait-only drain."""
    for f in nc.m.functions:
        for blk in f.blocks:
            insts = list(blk.instructions)
            out = []
            prev_by_engine = {}
            changed = False
            for inst in insts:
                si = inst.sync_info
                waits = list(si.on_wait) if si else []
                if len(waits) > max_waits:
                    changed = True
                    extra, keep = waits[:-max_waits], waits[-max_waits:]
                    remaining = []
                    prev = prev_by_engine.get(str(inst.engine))
                    for w in extra:
                        psi = prev.sync_info if prev is not None else None
                        if prev is not None and (
                            psi is None
                            or (len(psi.on_wait) == 0 and len(psi.on_update) == 0)
                        ):
                            prev.sync_info = mybir.SyncInfo(on_wait=[w], on_update=[])
                            prev = None  # one hoist per predecessor
                        else:
                            remaining.append(w)
                    for w in remaining:
                        dr = mybir.InstDrain(name=f"I-{nc.next_id()}", ins=[], outs=[])
                        dr.engine = inst.engine
                        dr.sync_info = mybir.SyncInfo(on_wait=[w], on_update=[])
                        out.append(dr)
                    inst.sync_info = mybir.SyncInfo(
                        on_wait=keep, on_update=list(si.on_update)
                    )
                out.append(inst)
                prev_by_engine[str(inst.engine)] = inst
            if changed:
                blk.instructions = out


def dedupe_ldweights(nc):
    """Drop an InstLdweights whose operand AP is byte-identical to the
    immediately preceding PE instruction's InstLdweights (no other PE
    instruction between them) -- the stationary operand is still loaded.
    Only legal when the elided load carries no sync actions."""
    for f in nc.m.functions:
        for blk in f.blocks:
            insts = list(blk.instructions)
            out = []
            last_pe_ldw_key = None
            changed = False
            for inst in insts:
                if str(inst.engine) != "EngineType.PE":
                    out.append(inst)
                    continue
                tname = type(inst).__name__
                if tname == "InstLdweights":
                    si = inst.sync_info
                    has_sync = si and (len(si.on_wait) or len(si.on_update))
                    try:
                        key = str(inst.ins[0])
                    except Exception:
                        key = None
                    if key is not None and key == last_pe_ldw_key and not has_sync:
                        changed = True
                        continue  # elide duplicate load
                    last_pe_ldw_key = key
                    out.append(inst)
                else:
                    if tname == "InstMatmult":
                        if getattr(inst, "is_transpose", None):
                            last_pe_ldw_key = None
                    else:
                        last_pe_ldw_key = None
                    out.append(inst)
            if changed:
                blk.instructions = out


def build_program():
    nc = bass.Bass("TRN2", target_bir_lowering=False, debug=False, num_devices=N_CORES)

    x_d = nc.dram_tensor("x", [BPC, L], F32, kind="ExternalInput")
    mqk_d = nc.dram_tensor("m_qk", [KA, KA], BF16, kind="ExternalInput")
    mvo_d = nc.dram_tensor("m_vo", [KA, KA], BF16, kind="ExternalInput")
    out_d = nc.dram_tensor("loss_partial", [1, 1], F32, kind="ExternalOutput")

    from contextlib import ExitStack

    with SplitDrainTileContext(nc) as tc, ExitStack() as ctx:
        cpool = ctx.enter_context(tc.tile_pool(name="consts", bufs=1))
        # PSUM: rotating pool (2x [128,1024] = 4 banks) for transient tiles;
        # persistent pool for pred_u + the small stats tiles (4 banks).
        prot = ctx.enter_context(tc.tile_pool(name="prot", bufs=2, space="PSUM"))
        ppu = ctx.enter_context(tc.tile_pool(name="ppu", bufs=2, space="PSUM"))
        xpool = ctx.enter_context(tc.tile_pool(name="xc", bufs=4))
        spool = ctx.enter_context(tc.tile_pool(name="small", bufs=8))
        bigpool = ctx.enter_context(tc.tile_pool(name="big", bufs=2))
        epool = ctx.enter_context(tc.tile_pool(name="et", bufs=3))
        scratch = ctx.enter_context(tc.tile_pool(name="scratch", bufs=2))

        # ---- constants ----
        ident_f = cpool.tile([128, 128], F32)
        make_identity(nc, ident_f[:])
        ident_b = cpool.tile([128, 128], BF16)
        make_identity(nc, ident_b[:])
        # doubled keep-mask (upper incl diag) for the DVE diagonal-block
        # mask of both batches at once
        triu2 = cpool.tile([128, 256], BF16)
        make_upper_triangular(nc, triu2[:, 0:128], val=1.0, diag=True)
        make_upper_triangular(nc, triu2[:, 128:256], val=1.0, diag=True)
        ones_col = cpool.tile([128, 1], F32)
        nc.vector.memset(ones_col[:], 1.0)
        ones_row = cpool.tile([1, PS], F32)
        nc.vector.memset(ones_row[:], 1.0)
        ones_t = cpool.tile([128, PS], BF16)
        nc.vector.memset(ones_t[:], 1.0)

        mqk2 = cpool.tile([128, KA], BF16)
        nc.gpsimd.dma_start(mqk2[0:KA, :], mqk_d.ap()[:])
        nc.gpsimd.dma_start(mqk2[PB : PB + KA, :], mqk_d.ap()[:])
        mvo2 = cpool.tile([128, KA], BF16)
        nc.gpsimd.dma_start(mvo2[0:KA, :], mvo_d.ap()[:])
        nc.gpsimd.dma_start(mvo2[PB : PB + KA, :], mvo_d.ap()[:])

        lp_all = cpool.tile([128, 4], F32)  # per-(pair, half) loss partials
        nc.vector.memset(lp_all[:], 0.0)


        # ---- x loads + per-batch sums + per-pair stats chains ----
        xcall = xpool.tile([128, BPC * (L // 128)], F32, name="xcall")
        CB = L // 128  # 256 cols per batch
        xcs = [xcall[:, b * CB : (b + 1) * CB] for b in range(BPC)]
        sums = spool.tile([128, 2 * BPC], F32, tag="sums")
        for b in range(BPC):
            # partition u, free (k, ps) <- x[b, (128k + u)*32 + ps]
            qeng = nc.sync if b % 2 == 0 else nc.scalar
            qeng.dma_start(
                xcall[:, b * CB : (b + 1) * CB].rearrange(
                    "u (k ps) -> u k ps", ps=PS
                ),
                x_d.ap()[b].rearrange("(k u ps) -> u k ps", u=128, ps=PS),
            )
            sc_col = 4 * (b // 2) + (b % 2)  # pair-contiguous: sA sB qA qB
            nc.vector.reduce_sum(sums[:, sc_col : sc_col + 1], xcs[b], axis=AX.X)
            sq_scr = scratch.tile([128, CB], F32, tag="sq", name=f"sq_{b}")
            nc.vector.tensor_tensor(out=sq_scr[:], in0=xcs[b], in1=xcs[b], op=ALU.mult)
            nc.vector.reduce_sum(
                sums[:, sc_col + 2 : sc_col + 3], sq_scr[:], axis=AX.X
            )

        # force the ACT table load (Ln/Exp set) right after the scalar
        # queue's x-load dispatches -- self-read, so it neither waits on the
        # lp memset nor delays the HWDGE dispatches ahead of it
        actw = cpool.tile([1, 2], F32)
        nc.scalar.activation(actw[:], actw[:], AF.Exp)

        scqs = []
        for p in range(2):
            # tot = (sumA, sumB, sqA, sqB) for this pair
            tot_ps = ppu.tile([1, 4], F32, tag="pu", name=f"totps_{p}")
            nc.tensor.matmul(
                tot_ps[:], ones_col[:], sums[:, 4 * p : 4 * p + 4],
                start=True, stop=True, skip_group_check=True,
            )
            tot = spool.tile([1, 4], F32, tag="tot", name=f"tot_{p}")
            nc.vector.tensor_copy(tot[:], tot_ps[:])
            w = spool.tile([1, 12], F32, tag="w", name=f"w_{p}")
            scq = spool.tile([1, 4], F32, tag="scq", name=f"scq_{p}")
            q_r = scq[:].rearrange("p (b q) -> p b q", q=2)[:, :, 0:1]
            q_s = scq[:].rearrange("p (b q) -> p b q", q=2)[:, :, 1:2]
            nc.scalar.mul(w[:, 0:2], tot[:, 0:2], 1.0 / L)  # mean
            nc.vector.tensor_tensor(
                out=w[:, 2:4], in0=tot[:, 0:2], in1=w[:, 0:2], op=ALU.mult
            )
            nc.vector.tensor_tensor(
                out=w[:, 4:6], in0=tot[:, 2:4], in1=w[:, 2:4], op=ALU.subtract
            )
            nc.scalar.activation(w[:, 6:8], w[:, 4:6], AF.Ln, scale=1.0 / (L - 1))
            # rstd = exp(-ln(var)/2); dropping the reference's +1e-5 on std
            # shifts rstd by 1e-5 relative -- far below the bf16 noise
            nc.scalar.activation(q_r, w[:, 6:8], AF.Exp, scale=-0.5)
            nc.scalar.mul(w[:, 0:2], w[:, 0:2], -1.0)  # -mean
            nc.vector.tensor_tensor(out=q_s, in0=w[:, 0:2], in1=q_r, op=ALU.mult)
            scqs.append(scq)

        ones_r128 = cpool.tile([1, 128], F32)
        nc.vector.memset(ones_r128[:], 1.0)
        bcp = []
        for p in range(2):
            # (rstd, shift) broadcast to all 128 partitions; A cols 0:2,
            # B cols 2:4 (the transposed layout mixes batches by column,
            # so each batch needs its scalars on every partition)
            bc_ps = ppu.tile([128, 4], F32, tag="pu", name=f"bcps_{p}")
            nc.tensor.matmul(
                bc_ps[:, 0:2], ones_r128[:], scqs[p][:, 0:2],
                start=True, stop=True, skip_group_check=True,
            )
            nc.tensor.matmul(
                bc_ps[:, 2:4], ones_r128[:], scqs[p][:, 2:4],
                start=True, stop=True, skip_group_check=True,
            )
            bc = spool.tile([128, 4], F32, tag="bc", name=f"bc_{p}")
            nc.vector.tensor_copy(bc[:], bc_ps[:])
            bcp.append(bc)

        # ---- per-pair state ----
        xnt = [None, None]
        y = [None, None]
        vw = [None, None]
        pu = [None, None]
        rr = [None, None]
        bcrs = [None, None]
        predt = [None, None]
        dds = [None, None]

        def prologue_tp(p):
            """Two whole-tile [128,128] PE transposes per batch (legal:
            full-partition output at base 0), one normalize per batch, then
            4 regroup DMAs per batch scatter the (kc,ps)-interleaved rows
            into xnt token order."""
            bc = bcp[p]
            xnt[p] = bigpool.tile([128, T], BF16, tag="xnt", name=f"xnt_{p}")
            xt = xnt[p]
            nc.vector.memset(xt[PS : PS + 1, :], 1.0)
            nc.vector.memset(xt[PB + PS : PB + PS + 1, :], 1.0)
            tp_ps = prot.tile([128, 512], F32, tag="rot", name=f"tp_{p}")
            for bi in range(2):
                for h in range(2):
                    nc.tensor.transpose(
                        tp_ps[:, 256 * bi + 128 * h : 256 * bi + 128 * h + 128],
                        xcs[2 * p + bi][:, 128 * h : 128 * h + 128],
                        ident_f[:],
                    )
            stag = scratch.tile([128, 512], BF16, tag="stag", name=f"stag_{p}")
            for bi in range(2):
                nc.vector.tensor_scalar(
                    out=stag[:, 256 * bi : 256 * bi + 256],
                    in0=tp_ps[:, 256 * bi : 256 * bi + 256],
                    scalar1=bc[:, 2 * bi : 2 * bi + 1],
                    scalar2=bc[:, 2 * bi + 1 : 2 * bi + 2],
                    op0=ALU.mult,
                    op1=ALU.add,
                )
                # row 32*kc+ps, col (h,u) of this batch block holds token
                # 128*(4h+kc)+u elem ps -> xnt[row, 128*(4h+kc)+u]
                row0 = 0 if bi == 0 else PB
                for kc in range(4):
                    qeng = (nc.sync, nc.scalar, nc.gpsimd, nc.sync)[kc]
                    qeng.dma_start(
                        xt[row0 : row0 + PS, :]
                        .rearrange("p (h k u) -> p h k u", k=4, u=128)[:, :, kc, :],
                        stag[
                            32 * kc : 32 * kc + PS, 256 * bi : 256 * bi + 256
                        ].rearrange("p (h u) -> p h u", u=128),
                    )

        def prologue_yvw(p):
            xt = xnt[p]
            # Y = M_qk^T Xa, pair-concurrent
            y[p] = bigpool.tile([128, T], BF16, tag="y", name=f"y_{p}")
            for n in range(2):
                y_ps = prot.tile([128, 512], F32, tag="rot", name=f"yps_{p}_{n}")
                nc.tensor.matmul(
                    y_ps[0:KA, :], mqk2[0:KA, :],
                    xt[0:KA, n * 512 : (n + 1) * 512],
                    start=True, stop=True, skip_group_check=True,
                )
                nc.tensor.matmul(
                    y_ps[PB : PB + KA, :], mqk2[PB : PB + KA, :],
                    xt[PB : PB + KA, n * 512 : (n + 1) * 512],
                    start=True, stop=True, skip_group_check=True,
                )
                nc.vector.tensor_copy(
                    y[p][0 : PB + KA, n * 512 : (n + 1) * 512], y_ps[0 : PB + KA, :]
                )

            # VW = Xa^T M_vo_aug: A_j in bank0 at 64j, B_j in bank1 (two
            # concurrent full-partition MMs must not share a PSUM bank)
            vw_ps = prot.tile([128, 1024], F32, tag="rot", name=f"vwps_{p}")
            for j in range(NT):
                nc.tensor.matmul(
                    vw_ps[:, 64 * j : 64 * j + KA],
                    xt[0:KA, j * 128 : (j + 1) * 128],
                    mvo2[0:KA, :],
                    start=True, stop=True, skip_group_check=True,
                )
                nc.tensor.matmul(
                    vw_ps[:, 512 + 64 * j : 512 + 64 * j + KA],
                    xt[PB : PB + KA, j * 128 : (j + 1) * 128],
                    mvo2[PB : PB + KA, :],
                    start=True, stop=True, skip_group_check=True,
                )
            # vw cols: A_j at 33j, B_j at 264+33j
            vw[p] = bigpool.tile([128, NT * 2 * KA], BF16, tag="vw", name=f"vw_{p}")
            nc.vector.tensor_copy(
                vw[p][:].rearrange("u (s e) -> u s e", e=KA),
                vw_ps[:].rearrange("u (s e) -> u s e", e=64)[:, :, 0:KA],
            )
            pu[p] = ppu.tile([128, 1024], F32, tag="pu", name=f"pu_{p}")
            rr[p] = scratch.tile([128, 1024], BF16, tag="rr", name=f"rr_{p}")
            bcrs[p] = scratch.tile([128, 1024], F32, tag="bcr", name=f"bcr_{p}")
            predt[p] = scratch.tile([128, 1024], BF16, tag="predt", name=f"predt_{p}")
            dds[p] = scratch.tile([128, 1024], BF16, tag="dd", name=f"dd_{p}")
            nc.gpsimd.memset(dds[p][PS:PB, 0 : T - 1], 0.0)

        def main_half(p, n):
            """scores -> exp -> PV for one t-half, pair-concurrent.
            pu checkerboard: (A,h) -> bank h, (B,h) -> bank 1-h."""
            xt, yp, vwp, pup = xnt[p], y[p], vw[p], pu[p]
            nj = 4 * n + 4
            bcol = (1 - n) * 512
            for j in range(nj):
                off = max(0, j * 128 - n * 512)
                diag = j * 128 >= n * 512
                sc_ps = prot.tile(
                    [128, 1024], F32, tag="rot", name=f"scps_{p}_{n}_{j}"
                )
                nc.tensor.matmul(
                    sc_ps[:, off:512],
                    xt[0:KA, j * 128 : (j + 1) * 128],
                    yp[0:KA, n * 512 + off : (n + 1) * 512],
                    start=True, stop=True, skip_group_check=True,
                )
                nc.tensor.matmul(
                    sc_ps[:, 512 + off : 1024],
                    xt[PB : PB + KA, j * 128 : (j + 1) * 128],
                    yp[PB : PB + KA, n * 512 + off : (n + 1) * 512],
                    start=True, stop=True, skip_group_check=True,
                )
                et = epool.tile([128, 1024], BF16, tag="et", name=f"et_{p}_{n}_{j}")
                nc.scalar.activation(
                    et[:].rearrange("u (b c) -> u b c", b=2)[:, :, off:512],
                    sc_ps[:].rearrange("u (b c) -> u b c", b=2)[:, :, off:512],
                    AF.Exp,
                    scale=SCALE,
                )
                if diag:
                    # zero the s > t half of the diagonal block (DVE has
                    # slack in the stream; keeps the PE free)
                    db = et[:].rearrange("u (b c) -> u b c", b=2)[
                        :, :, off : off + 128
                    ]
                    nc.vector.tensor_tensor(
                        out=db, in0=db,
                        in1=triu2[:].rearrange("u (b c) -> u b c", b=2),
                        op=ALU.mult,
                    )
                nc.tensor.matmul(
                    pup[0:KA, n * 512 + off : (n + 1) * 512],
                    vwp[:, j * KA : (j + 1) * KA],
                    et[:, off:512],
                    start=(j == 0), stop=(j == nj - 1), skip_group_check=True,
                )
                nc.tensor.matmul(
                    pup[PB : PB + KA, bcol + off : bcol + 512],
                    vwp[:, NT * KA + j * KA : NT * KA + (j + 1) * KA],
                    et[:, 512 + off : 1024],
                    start=(j == 0), stop=(j == nj - 1), skip_group_check=True,
                )

        def epiA(p, n):
            """1/colsum for one half: css rows (32, A-cols) and (96, B-cols)
            DMA-gathered to a [128, 8] layout, DVE reciprocal, DMA back."""
            pup = pu[p]
            bcol = (1 - n) * 512
            csb = scratch.tile([128, 1024], F32, tag="lnr", name=f"csb_{p}_{n}")
            nc.vector.tensor_copy(csb[0 : PB + PS + 1, :], pup[0 : PB + PS + 1, :])
            stg = spool.tile([128, 8], F32, tag="stg", name=f"stg_{p}_{n}")
            nc.sync.dma_start(
                stg[:, 0:4].rearrange("p q -> p () q"),
                csb[PS : PS + 1, n * 512 : (n + 1) * 512].rearrange(
                    "p (a q) -> p a q", q=4
                ),
            )
            nc.scalar.dma_start(
                stg[:, 4:8].rearrange("p q -> p () q"),
                csb[PB + PS : PB + PS + 1, bcol : bcol + 512].rearrange(
                    "p (a q) -> p a q", q=4
                ),
            )
            rstg = spool.tile([128, 8], F32, tag="rstg", name=f"rstg_{p}_{n}")
            nc.vector.reciprocal(rstg[:], stg[:])
            rb16 = spool.tile([128, 8], BF16, tag="rb16", name=f"rb16_{p}_{n}")
            nc.vector.tensor_copy(rb16[:], rstg[:])
            rrp = rr[p]
            nc.sync.dma_start(
                rrp[PS : PS + 1, n * 512 : (n + 1) * 512].rearrange(
                    "p (a q) -> p a q", q=4
                ),
                rb16[:, 0:4].rearrange("p q -> p () q"),
            )
            nc.scalar.dma_start(
                rrp[PB + PS : PB + PS + 1, bcol : bcol + 512].rearrange(
                    "p (a q) -> p a q", q=4
                ),
                rb16[:, 4:8].rearrange("p q -> p () q"),
            )
        def epiA_scalar(p, n):
            """Same as epiA but via ScalarE Ln/Exp straight from PSUM --
            no DMA round-trip latency; used for the final half where the
            DMA latency cannot be hidden."""
            pup = pu[p]
            bcol = (1 - n) * 512
            rrp = rr[p]
            lnr = scratch.tile([128, 1024], F32, tag="lnr", name=f"lnr_{p}_{n}")
            # single-row ops: ACT cost is free-size-bound, and touching only
            # the css rows avoids a false WAW against the other half's rr
            # rows (which used to stall its broadcast MMs ~3.5us)
            nc.scalar.activation(
                lnr[PS : PS + 1, n * 512 : (n + 1) * 512],
                pup[PS : PS + 1, n * 512 : (n + 1) * 512],
                AF.Ln,
            )
            nc.scalar.activation(
                rrp[PS : PS + 1, n * 512 : (n + 1) * 512],
                lnr[PS : PS + 1, n * 512 : (n + 1) * 512],
                AF.Exp, scale=-1.0,
            )
            nc.scalar.activation(
                lnr[PB + PS : PB + PS + 1, bcol : bcol + 512],
                pup[PB + PS : PB + PS + 1, bcol : bcol + 512],
                AF.Ln,
            )
            nc.scalar.activation(
                rrp[PB + PS : PB + PS + 1, bcol : bcol + 512],
                lnr[PB + PS : PB + PS + 1, bcol : bcol + 512],
                AF.Exp, scale=-1.0,
            )

        def epiB(p, n):
            """PE broadcast of 1/css + evacuation to sbuf (emitted late so
            the PE FIFO never blocks on the recip DMA chain)."""
            bcol = (1 - n) * 512
            rrp = rr[p]
            bcr_ps = prot.tile([128, 1024], F32, tag="rot", name=f"bcrps_{p}_{n}")
            nc.tensor.matmul(
                bcr_ps[0:PS, n * 512 : (n + 1) * 512],
                ones_t[PS : PS + 1, :],
                rrp[PS : PS + 1, n * 512 : (n + 1) * 512],
                start=True, stop=True, skip_group_check=True,
            )
            nc.tensor.matmul(
                bcr_ps[PB : PB + PS, bcol : bcol + 512],
                ones_t[PB + PS : PB + PS + 1, :],
                rrp[PB + PS : PB + PS + 1, bcol : bcol + 512],
                start=True, stop=True, skip_group_check=True,
                tile_position=(PB + PS, PB),
            )
            ceng = nc.scalar if (p, n) in ((0, 1), (1, 1)) else None
            if ceng is not None:
                ceng.copy(
                    bcrs[p][0:PS, n * 512 : (n + 1) * 512],
                    bcr_ps[0:PS, n * 512 : (n + 1) * 512],
                )
                ceng.copy(
                    bcrs[p][PB : PB + PS, bcol : bcol + 512],
                    bcr_ps[PB : PB + PS, bcol : bcol + 512],
                )
            else:
                nc.vector.tensor_copy(
                    bcrs[p][0:PS, n * 512 : (n + 1) * 512],
                    bcr_ps[0:PS, n * 512 : (n + 1) * 512],
                )
                nc.vector.tensor_copy(
                    bcrs[p][PB : PB + PS, bcol : bcol + 512],
                    bcr_ps[PB : PB + PS, bcol : bcol + 512],
                )

        def tail_half(p, n):
            """pred = pu/css and squared-error partial for one t-half.
            dd col c holds t=c; the B rows read checkerboarded pred cols."""
            pup, xt, pt, dd = pu[p], xnt[p], predt[p], dds[p]
            bcol = (1 - n) * 512
            nc.vector.tensor_tensor(
                out=pt[0:PS, n * 512 : (n + 1) * 512],
                in0=pup[0:PS, n * 512 : (n + 1) * 512],
                in1=bcrs[p][0:PS, n * 512 : (n + 1) * 512],
                op=ALU.mult,
            )
            nc.vector.tensor_tensor(
                out=pt[PB : PB + PS, bcol : bcol + 512],
                in0=pup[PB : PB + PS, bcol : bcol + 512],
                in1=bcrs[p][PB : PB + PS, bcol : bcol + 512],
                op=ALU.mult,
            )
            # dd cols for this half: t in [512n, 512n+512) (clip t=1023)
            c0 = n * 512
            c1 = min((n + 1) * 512, T - 1)
            deng = nc.gpsimd if p == 0 else nc.vector
            deng.tensor_tensor(
                out=dd[0:PS, c0:c1],
                in0=pt[0:PS, c0:c1],
                in1=xt[0:PS, c0 + 1 : c1 + 1],
                op=ALU.subtract,
            )
            deng.tensor_tensor(
                out=dd[PB : PB + PS, c0:c1],
                in0=pt[PB : PB + PS, bcol : bcol + (c1 - c0)],
                in1=xt[PB : PB + PS, c0 + 1 : c1 + 1],
                op=ALU.subtract,
            )
            nc.scalar.activation(
                dd[0 : PB + PS, c0:c1],
                dd[0 : PB + PS, c0:c1],
                AF.Square,
                accum_out=lp_all[0 : PB + PS, 2 * p + n : 2 * p + n + 1],
            )

        # ---- software-pipelined emission ----
        prologue_tp(0)
        prologue_tp(1)
        prologue_yvw(0)
        prologue_yvw(1)
        main_half(0, 0)
        epiA(0, 0)
        main_half(0, 1)
        epiA(0, 1)
        main_half(1, 0)
        epiA(1, 0)
        epiB(0, 0)
        tail_half(0, 0)
        main_half(1, 1)
        epiB(0, 1)
        tail_half(0, 1)
        epiA_scalar(1, 1)
        epiB(1, 0)
        tail_half(1, 0)
        epiB(1, 1)
        tail_half(1, 1)

        # ---- final: total partial over pairs & partitions ----
        lsum = spool.tile([128, 1], F32)
        nc.vector.reduce_sum(lsum[:], lp_all[:], axis=AX.X)
        tot_ps2 = prot.tile([1, 1], F32, tag="rot")
        nc.tensor.matmul(tot_ps2[:], ones_col[:], lsum[:], start=True, stop=True)
        out_sb = spool.tile([1, 1], F32)
        nc.vector.tensor_copy(out_sb[:], tot_ps2[:])
        nc.sync.dma_start(out_d.ap()[:], out_sb[:])

    split_excess_waits(nc)
    dedupe_ldweights(nc)
    return nc


_program_cache = {}


def _get_program():
    if "nc" not in _program_cache:
        _program_cache["nc"] = build_program()
    return _program_cache["nc"]


def make_in_maps(x, W_proj, b_proj, W_qkv, b_qkv, W_out, b_out, W_head, b_head):
    import ml_dtypes

    f8 = np.float64
    w_eff = W_proj.astype(f8) @ W_qkv.astype(f8)  # [32, 768]
    b_eff = b_proj.astype(f8) @ W_qkv.astype(f8) + b_qkv.astype(f8)  # [768]
    w_aug = np.concatenate([w_eff, b_eff[None, :]], axis=0)  # [33, 768]
    wq, wk, wv = w_aug[:, 0:D], w_aug[:, D : 2 * D], w_aug[:, 2 * D : 3 * D]
    m_qk = wq @ wk.T  # [33, 33]
    w_oh = W_out.astype(f8) @ W_head.astype(f8)  # [256, 32]
    b_oh = b_out.astype(f8) @ W_head.astype(f8) + b_head.astype(f8)  # [32]
    m_vo = wv @ w_oh  # [33, 32]
    m_vo[PS, :] += b_oh
    e_ones = np.zeros((KA, 1), f8)
    e_ones[PS, 0] = 1.0  # selects Xa's ones row -> colsum output column
    m_vo_aug = np.concatenate([m_vo, e_ones], axis=1)  # [33, 33]

    mqk_b = np.ascontiguousarray(m_qk.astype(ml_dtypes.bfloat16))
    mvo_b = np.ascontiguousarray(m_vo_aug.astype(ml_dtypes.bfloat16))

    in_maps = []
    for core in range(N_CORES):
        xs = np.ascontiguousarray(x[core * BPC : (core + 1) * BPC])
        in_maps.append({"x": xs, "m_qk": mqk_b, "m_vo": mvo_b})
    return in_maps


def kernel(**inputs) -> np.ndarray:
    inputs = {k: np.asarray(v) for k, v in inputs.items()}
    nc = _get_program()
    in_maps = make_in_maps(**inputs)
    res = run_bass_kernel_spmd(nc, in_maps, core_ids=list(range(N_CORES)))
    total = sum(float(res.results[i]["loss_partial"][0, 0]) for i in range(N_CORES))
    loss = total / (B * (T - 1) * PS)
    return np.float32(loss)


if __name__ == "__main__":
    rng = np.random.default_rng(0)
    ins = {
        "x": rng.standard_normal((B, L)).astype(np.float32),
        "W_proj": (rng.standard_normal((PS, D)) / math.sqrt(PS)).astype(np.float32),
        "b_proj": np.zeros(D, np.float32),
        "W_qkv": (rng.standard_normal((D, 3 * D)) / math.sqrt(D)).astype(np.float32),
        "b_qkv": np.zeros(3 * D, np.float32),
        "W_out": (rng.standard_normal((D, D)) / math.sqrt(D)).astype(np.float32),
        "b_out": np.zeros(D, np.float32),
        "W_head": (rng.standard_normal((D, PS)) / math.sqrt(D)).astype(np.float32),
        "b_head": np.zeros(PS, np.float32),
    }
    got = kernel(**ins)
    print("kernel loss:", got)

